# revision 1
# baseline (speedup 1.0000x reference)
"""Adaptive-softmax NLL loss on 8 TRN2 NeuronCores.

Strategy: tensor-parallel over the vocab dimension. Each core computes the
exp-sums of its vocab slice of head / tail1 / tail2 logits for all 4096
tokens, plus (token-sharded) the gathered target-logit dot products. One
small AllReduce combines per-token sum-exp partials; every core then
finishes the scalar NLL identically.

NLL = sum_n log(S_head_n) + sum_{n in c1} log(S_t1_n) + sum_{n in c2} log(S_t2_n)
      - sum_n x_n . W_ext[cidx_n] - sum_{c1} h1_n . W1[t_n-C0] - sum_{c2} h2_n . W2[t_n-C1]

where S_* are softmax denominators (no max-subtraction needed: logits are
O(1) by construction), cidx_n = target_n if < C0 else C0/C0+1 (cluster
prior column), and h1/h2 are the low-rank projections computed on device.
Host-side work is limited to index gathers / layout transforms of inputs.
"""

import os
import sys

for _p in ("/opt/trn_rl_repo",):
    if _p not in sys.path:
        sys.path.insert(0, _p)

import numpy as np

import concourse.bacc as bacc
import concourse.bass as bass
import concourse.mybir as mybir
import concourse.tile as tile
from concourse.bass_utils import run_bass_kernel_spmd

dt = mybir.dt
AF = mybir.ActivationFunctionType
ALU = mybir.AluOpType

NCORES = 8
N, D = 4096, 1024
C0, C1, C2 = 20000, 40000, 50257
VH = C0 + 2          # head logits incl 2 cluster columns
R1, R2 = 256, 64
VHC = 2560           # head vocab rows per core (8*2560 = 20480, pad 478)
V1C = 2560           # tail1 rows per core   (8*2560 = 20480, pad 480)
V2C = 1536           # tail2 rows per core   (8*1536 = 12288, pad 2031)
PAD_H = NCORES * VHC - VH
PAD_1 = NCORES * V1C - (C1 - C0)
PAD_2 = NCORES * V2C - (C2 - C1)
NT = N // 128        # 32 token tiles
NSH = N // NCORES    # 512 tokens per core for the sharded head dot

F32, F32R = dt.float32, dt.float32r

LAST_EXEC_NS = None


def _build(phases=4):
    nc = bacc.Bacc("TRN2", target_bir_lowering=False, debug=False,
                   num_devices=NCORES)

    xT = nc.declare_dram_parameter("xT", [D, N], F32, isOutput=False)
    whT = nc.declare_dram_parameter("whT", [D, VHC], F32, isOutput=False)
    w1T = nc.declare_dram_parameter("w1T", [R1, V1C], F32, isOutput=False)
    w2T = nc.declare_dram_parameter("w2T", [R2, V2C], F32, isOutput=False)
    p1T = nc.declare_dram_parameter("p1T", [D, R1], F32, isOutput=False)
    p2T = nc.declare_dram_parameter("p2T", [D, R2], F32, isOutput=False)
    xTc = nc.declare_dram_parameter("xTc", [D, NSH], F32, isOutput=False)
    wselT = nc.declare_dram_parameter("wselT", [D, NSH], F32, isOutput=False)
    w1selT = nc.declare_dram_parameter("w1selT", [R1, N], F32, isOutput=False)
    w2selT = nc.declare_dram_parameter("w2selT", [R2, N], F32, isOutput=False)
    m1_in = nc.declare_dram_parameter("m1", [128, NT], F32, isOutput=False)
    m2_in = nc.declare_dram_parameter("m2", [128, NT], F32, isOutput=False)
    out_ext = nc.declare_dram_parameter("out", [1, 1], F32, isOutput=True)

    KD = D // 128  # 8 k-tiles over the D contraction

    with tile.TileContext(nc) as tc:
        with (
            tc.tile_pool(name="res", bufs=1) as res,       # resident tensors
            tc.tile_pool(name="dram", bufs=1, space="DRAM") as dram,
        ):
            # ---- resident loads -------------------------------------------------
            whT_sb = res.tile([128, KD * VHC], F32R)   # [p, (k v)]
            nc.sync.dma_start(
                out=whT_sb[:].rearrange("p (k v) -> p k v", k=KD),
                in_=whT.ap().bitcast(F32R).rearrange("(k p) v -> p k v", p=128))
            w1T_sb = res.tile([128, 2 * V1C], F32R)
            nc.sync.dma_start(
                out=w1T_sb[:].rearrange("p (k v) -> p k v", k=2),
                in_=w1T.ap().bitcast(F32R).rearrange("(k p) v -> p k v", p=128))
            w2T_sb = res.tile([64, V2C], F32R)
            nc.sync.dma_start(out=w2T_sb[:], in_=w2T.ap().bitcast(F32R))
            p1T_sb = res.tile([128, KD * R1], F32R)
            nc.sync.dma_start(
                out=p1T_sb[:].rearrange("p (k r) -> p k r", k=KD),
                in_=p1T.ap().bitcast(F32R).rearrange("(k p) r -> p k r", p=128))
            p2T_sb = res.tile([128, KD * R2], F32R)
            nc.sync.dma_start(
                out=p2T_sb[:].rearrange("p (k r) -> p k r", k=KD),
                in_=p2T.ap().bitcast(F32R).rearrange("(k p) r -> p k r", p=128))
            m1_sb = res.tile([128, NT], F32)
            nc.sync.dma_start(out=m1_sb[:], in_=m1_in.ap())
            m2_sb = res.tile([128, NT], F32)
            nc.sync.dma_start(out=m2_sb[:], in_=m2_in.ap())

            h1T_sb = [res.tile([128, N], F32R, tag=f"h1T{r}", name=f"h1T{r}")
                      for r in range(2)]
            h2T_sb = res.tile([64, N], F32R)

            sh_slots = res.tile([128, NT], F32)
            s1_slots = res.tile([128, NT], F32)
            s2_slots = res.tile([128, NT], F32)
            dsh_slots = res.tile([128, KD], F32)   # sharded head dot partials
            dgl_slots = res.tile([128, 8], F32)    # unsharded t1/t2 dot partials
            nc.vector.memset(dgl_slots[:], 0.0)

            # ---- phase 1: projections h1T = P1 @ x.T, h2T = P2 @ x.T ------------
            with tc.tile_pool(name="pj", bufs=1, space="PSUM") as pj, \
                 tc.tile_pool(name="s1p", bufs=3) as stream:
                for q in range(4):           # token quarters of 1024
                    pa = pj.tile([128, 1024], F32, tag="pa")
                    pb = pj.tile([128, 1024], F32, tag="pb")
                    pc = pj.tile([64, 1024], F32, tag="pc")
                    for k in range(KD):
                        xq = stream.tile([128, 1024], F32R, tag="xq")
                        nc.sync.dma_start(
                            out=xq[:],
                            in_=xT.ap().bitcast(F32R)[k * 128:(k + 1) * 128,
                                        q * 1024:(q + 1) * 1024])
                        st = dict(start=(k == 0), stop=(k == KD - 1))
                        for h in range(2):
                            sl = slice(h * 512, (h + 1) * 512)
                            nc.tensor.matmul(
                                pa[:, sl],
                                lhsT=p1T_sb[:, k * R1:k * R1 + 128],
                                rhs=xq[:, sl], **st)
                            nc.tensor.matmul(
                                pb[:, sl],
                                lhsT=p1T_sb[:, k * R1 + 128:(k + 1) * R1],
                                rhs=xq[:, sl], **st)
                            nc.tensor.matmul(
                                pc[:, sl],
                                lhsT=p2T_sb[:, k * R2:(k + 1) * R2],
                                rhs=xq[:, sl], **st)
                    qs = slice(q * 1024, (q + 1) * 1024)
                    nc.vector.tensor_copy(h1T_sb[0][:, qs], pa[:])
                    nc.vector.tensor_copy(h1T_sb[1][:, qs], pb[:])
                    nc.vector.tensor_copy(h2T_sb[:, qs], pc[:])

            if phases == 1:
                # debug: reduce h1T/h2T to a scalar-ish output and stop
                dbg = res.tile([128, 1], F32)
                nc.vector.reduce_sum(dbg[:], h1T_sb[0][:].bitcast(F32),
                                     axis=mybir.AxisListType.X)
                out_sb1 = res.tile([1, 1], F32, name="dbg_out", uniquify=True)
                nc.gpsimd.tensor_reduce(out_sb1[:], dbg[:],
                                        axis=mybir.AxisListType.C,
                                        op=ALU.add)
                nc.sync.dma_start(out=out_ext.ap(), in_=out_sb1[:])

            if phases >= 2:
                # ---- phase 2: head + tail logits, exp, per-token sum-exp ------------
                NVC_H = VHC // 512   # 5 chunks of 512
                NVC_1 = V1C // 512   # 5
                NVC_2 = V2C // 512   # 3
                with tc.tile_pool(name="p2", bufs=1, space="PSUM") as p2p, \
                     tc.tile_pool(name="s2p", bufs=3) as stream:
                    for nt in range(NT):
                        xnt = stream.tile([128, KD * 128], F32R, tag="xnt")
                        nc.sync.dma_start(
                            out=xnt[:].rearrange("p (k c) -> p k c", k=KD),
                            in_=xT.ap().bitcast(F32R)
                                .rearrange("(k p) n -> p k n", p=128)
                                [:, :, nt * 128:(nt + 1) * 128])

                        ph = p2p.tile([128, VHC], F32, tag="big")
                        for k in range(KD):
                            lhs = xnt[:, k * 128:(k + 1) * 128]
                            st = dict(start=(k == 0), stop=(k == KD - 1))
                            for vc in range(NVC_H):
                                nc.tensor.matmul(
                                    ph[:, vc * 512:(vc + 1) * 512], lhsT=lhs,
                                    rhs=whT_sb[:, k * VHC + vc * 512:
                                               k * VHC + (vc + 1) * 512],
                                    **st)
                        # tail2 (3 banks) runs while exp(head) drains
                        pt2 = p2p.tile([128, V2C], F32, tag="pt2")
                        for vc in range(NVC_2):
                            nc.tensor.matmul(
                                pt2[:, vc * 512:(vc + 1) * 512],
                                lhsT=h2T_sb[:, nt * 128:(nt + 1) * 128],
                                rhs=w2T_sb[:, vc * 512:(vc + 1) * 512],
                                start=True, stop=True)
                        nc.scalar.activation(ph[:], ph[:], AF.Exp,
                                             accum_out=sh_slots[:, nt:nt + 1])
                        pt1 = p2p.tile([128, V1C], F32, tag="big")
                        for k in range(2):
                            lhs = h1T_sb[k][:, nt * 128:(nt + 1) * 128]
                            st = dict(start=(k == 0), stop=(k == 1))
                            for vc in range(NVC_1):
                                nc.tensor.matmul(
                                    pt1[:, vc * 512:(vc + 1) * 512], lhsT=lhs,
                                    rhs=w1T_sb[:, k * V1C + vc * 512:
                                               k * V1C + (vc + 1) * 512],
                                    **st)
                        nc.scalar.activation(pt2[:], pt2[:], AF.Exp,
                                             accum_out=s2_slots[:, nt:nt + 1])
                        nc.scalar.activation(pt1[:], pt1[:], AF.Exp,
                                             accum_out=s1_slots[:, nt:nt + 1])

            if phases == 2:
                dbg = res.tile([128, 1], F32)
                nc.vector.reduce_sum(dbg[:], sh_slots[:],
                                     axis=mybir.AxisListType.X)
                dbg2 = res.tile([128, 1], F32)
                nc.vector.reduce_sum(dbg2[:], s1_slots[:],
                                     axis=mybir.AxisListType.X)
                nc.vector.tensor_add(dbg[:], dbg[:], dbg2[:])
                nc.vector.reduce_sum(dbg2[:], s2_slots[:],
                                     axis=mybir.AxisListType.X)
                nc.vector.tensor_add(dbg[:], dbg[:], dbg2[:])
                out_sb1 = res.tile([1, 1], F32, name="dbg_out", uniquify=True)
                nc.gpsimd.tensor_reduce(out_sb1[:], dbg[:],
                                        axis=mybir.AxisListType.C,
                                        op=ALU.add)
                nc.sync.dma_start(out=out_ext.ap(), in_=out_sb1[:])

            if phases >= 3:
                # ---- phase 3: gathered-logit dot products (DVE) ---------------------
                dotp = tc.tile_pool(name="s3p", bufs=2)
                stream = dotp.__enter__()
                for k in range(KD):
                    xc = stream.tile([128, NSH], F32, tag="xc")
                    nc.sync.dma_start(out=xc[:],
                                      in_=xTc.ap()[k * 128:(k + 1) * 128, :])
                    wc = stream.tile([128, NSH], F32, tag="wc")
                    nc.sync.dma_start(out=wc[:],
                                      in_=wselT.ap()[k * 128:(k + 1) * 128, :])
                    scr = stream.tile([128, NSH], F32, tag="dscr", bufs=2)
                    nc.vector.tensor_mul(scr[:], xc[:], wc[:])
                    nc.vector.reduce_sum(dsh_slots[:, k:k + 1], scr[:],
                                         axis=mybir.AxisListType.X)
                CH = 1024
                for k in range(2):
                    for h in range(4):
                        w1c = stream.tile([128, CH], F32, tag="w1c")
                        nc.sync.dma_start(
                            out=w1c[:],
                            in_=w1selT.ap()[k * 128:(k + 1) * 128,
                                            h * CH:(h + 1) * CH])
                        scr2 = stream.tile([128, CH], F32, tag="dscr2", bufs=2)
                        nc.vector.tensor_mul(
                            scr2[:], h1T_sb[k][:, h * CH:(h + 1) * CH].bitcast(F32),
                            w1c[:])
                        nc.vector.reduce_sum(
                            dgl_slots[:, 4 * k + h:4 * k + h + 1], scr2[:],
                            axis=mybir.AxisListType.X)
                t2_slots = res.tile([64, 4], F32)
                nc.vector.memset(t2_slots[:], 0.0)
                for h in range(4):
                    w2c = stream.tile([64, CH], F32, tag="w2c")
                    nc.sync.dma_start(
                        out=w2c[:],
                        in_=w2selT.ap()[:, h * CH:(h + 1) * CH])
                    scr3 = stream.tile([64, CH], F32, tag="dscr2", bufs=2)
                    nc.vector.tensor_mul(
                        scr3[:], h2T_sb[:, h * CH:(h + 1) * CH].bitcast(F32),
                        w2c[:])
                    nc.vector.reduce_sum(t2_slots[:, h:h + 1], scr3[:],
                                         axis=mybir.AxisListType.X)

                dotp.__exit__(None, None, None)
                dsh_red = res.tile([128, 1], F32)
                nc.vector.reduce_sum(dsh_red[:], dsh_slots[:],
                                     axis=mybir.AxisListType.X)

            if phases == 3:
                dbg = res.tile([128, 1], F32)
                nc.vector.reduce_sum(dbg[:], dsh_slots[:],
                                     axis=mybir.AxisListType.X)
                dbg2 = res.tile([128, 1], F32)
                nc.vector.reduce_sum(dbg2[:], dgl_slots[:],
                                     axis=mybir.AxisListType.X)
                nc.vector.tensor_add(dbg[:], dbg[:], dbg2[:])
                out_sb1 = res.tile([1, 1], F32, name="dbg_out", uniquify=True)
                nc.gpsimd.tensor_reduce(out_sb1[:], dbg[:],
                                        axis=mybir.AxisListType.C,
                                        op=ALU.add)
                nc.sync.dma_start(out=out_ext.ap(), in_=out_sb1[:])

            if phases >= 4:
                # ---- phase 4: AllReduce of sum-exp partials + sharded head dot ------
                PAY = 3 * NT + 1
                pay_sb = res.tile([128, PAY], F32)
                nc.vector.tensor_copy(pay_sb[:, 0:NT], sh_slots[:])
                nc.vector.tensor_copy(pay_sb[:, NT:2 * NT], s1_slots[:])
                nc.vector.tensor_copy(pay_sb[:, 2 * NT:3 * NT], s2_slots[:])
                nc.vector.tensor_copy(pay_sb[:, 3 * NT:PAY], dsh_red[:])
                pay_dram = dram.tile([128, PAY], F32)
                red_dram = dram.tile([128, PAY], F32)
                nc.sync.dma_start(out=pay_dram[:], in_=pay_sb[:])
                nc.gpsimd.collective_compute(
                    "AllReduce", ALU.add,
                    replica_groups=[list(range(NCORES))],
                    ins=[pay_dram.opt()], outs=[red_dram.opt()])
                red_sb = res.tile([128, PAY], F32)
                nc.sync.dma_start(out=red_sb[:], in_=red_dram[:])

                # ---- phase 5: finish scalar NLL (identical on every core) -----------
                sadj = res.tile([128, 3 * NT], F32)
                nc.vector.tensor_scalar_add(sadj[:, 0:NT], red_sb[:, 0:NT],
                                            float(-PAD_H))
                nc.vector.tensor_scalar_add(sadj[:, NT:2 * NT],
                                            red_sb[:, NT:2 * NT], float(-PAD_1))
                nc.vector.tensor_scalar_add(sadj[:, 2 * NT:3 * NT],
                                            red_sb[:, 2 * NT:3 * NT], float(-PAD_2))
                logs = res.tile([128, 3 * NT], F32)
                nc.scalar.activation(logs[:], sadj[:], AF.Ln)
                lse = res.tile([128, NT], F32)
                nc.vector.tensor_mul(lse[:], logs[:, NT:2 * NT], m1_sb[:])
                t2m = res.tile([128, NT], F32)
                nc.vector.tensor_mul(t2m[:], logs[:, 2 * NT:3 * NT], m2_sb[:])
                nc.vector.tensor_add(lse[:], lse[:], logs[:, 0:NT])
                nc.vector.tensor_add(lse[:], lse[:], t2m[:])
                tot = res.tile([128, 1], F32)
                nc.vector.reduce_sum(tot[:], lse[:], axis=mybir.AxisListType.X)
                nc.vector.tensor_sub(tot[:], tot[:], red_sb[:, 3 * NT:PAY])
                dgr = res.tile([128, 1], F32)
                nc.vector.reduce_sum(dgr[:], dgl_slots[:],
                                     axis=mybir.AxisListType.X)
                nc.vector.tensor_sub(tot[:], tot[:], dgr[:])
                t2r = res.tile([64, 1], F32)
                nc.vector.reduce_sum(t2r[:], t2_slots[:],
                                     axis=mybir.AxisListType.X)
                nc.vector.tensor_sub(tot[:64, :], tot[:64, :], t2r[:])
                out_sb = res.tile([1, 1], F32)
                nc.gpsimd.tensor_reduce(out_sb[:], tot[:],
                                        axis=mybir.AxisListType.C, op=ALU.add)
                nc.sync.dma_start(out=out_ext.ap(), in_=out_sb[:])

    nc.compile()
    return nc


_NC = None


def _get_nc():
    global _NC
    if _NC is None:
        _NC = _build(phases=int(os.environ.get("KERNEL_PHASES", "4")))
    return _NC


def kernel(**inputs):
    x = np.ascontiguousarray(inputs["x"], dtype=np.float32)
    target = np.asarray(inputs["target"]).astype(np.int64)
    W_head = np.asarray(inputs["W_head"], dtype=np.float32)
    W_cluster = np.asarray(inputs["W_cluster"], dtype=np.float32)
    P1 = np.asarray(inputs["P1"], dtype=np.float32)
    W1 = np.asarray(inputs["W1"], dtype=np.float32)
    P2 = np.asarray(inputs["P2"], dtype=np.float32)
    W2 = np.asarray(inputs["W2"], dtype=np.float32)

    # ---- host-side sharding / index gathers (no arithmetic on values) ------
    W_ext = np.concatenate([W_head, W_cluster], axis=0)          # [20002, D]
    mask1 = (target >= C0) & (target < C1)
    mask2 = target >= C1
    cidx = np.where(target < C0, target,
                    np.where(mask1, C0, C0 + 1)).astype(np.int64)
    # gather from zero-padded matrices so out-of-cluster rows contribute 0
    W1p = np.concatenate([W1, np.zeros((1, R1), np.float32)], axis=0)
    W2p = np.concatenate([W2, np.zeros((1, R2), np.float32)], axis=0)
    j1 = np.where(mask1, target - C0, C1 - C0).astype(np.int64)
    j2 = np.where(mask2, target - C1, C2 - C1).astype(np.int64)

    xT = np.ascontiguousarray(x.T)                               # [D, N]
    WhT_full = np.zeros((NCORES * VHC, D), np.float32)
    WhT_full[:VH] = W_ext
    W1_full = np.zeros((NCORES * V1C, R1), np.float32)
    W1_full[:C1 - C0] = W1
    W2_full = np.zeros((NCORES * V2C, R2), np.float32)
    W2_full[:C2 - C1] = W2
    p1T = np.ascontiguousarray(P1.T)
    p2T = np.ascontiguousarray(P2.T)
    wselT = np.ascontiguousarray(W_ext[cidx].T)                  # [D, N]
    w1selT = np.ascontiguousarray(W1p[j1].T)                     # [R1, N]
    w2selT = np.ascontiguousarray(W2p[j2].T)                     # [R2, N]
    m1 = np.ascontiguousarray(
        mask1.astype(np.float32).reshape(NT, 128).T)             # [128, NT]
    m2 = np.ascontiguousarray(
        mask2.astype(np.float32).reshape(NT, 128).T)

    in_maps = []
    for i in range(NCORES):
        in_maps.append({
            "xT": xT,
            "whT": np.ascontiguousarray(
                WhT_full[i * VHC:(i + 1) * VHC].T),
            "w1T": np.ascontiguousarray(
                W1_full[i * V1C:(i + 1) * V1C].T),
            "w2T": np.ascontiguousarray(
                W2_full[i * V2C:(i + 1) * V2C].T),
            "p1T": p1T,
            "p2T": p2T,
            "xTc": np.ascontiguousarray(xT[:, i * NSH:(i + 1) * NSH]),
            "wselT": np.ascontiguousarray(wselT[:, i * NSH:(i + 1) * NSH]),
            "w1selT": w1selT,
            "w2selT": w2selT,
            "m1": m1,
            "m2": m2,
        })

    nc = _get_nc()
    trace = bool(int(os.environ.get("KERNEL_TRACE", "0")))
    if trace:
        _install_ntff_hook()
    res = run_bass_kernel_spmd(nc, in_maps, core_ids=list(range(NCORES)),
                               trace=trace)
    global LAST_EXEC_NS
    LAST_EXEC_NS = res.exec_time_ns
    val = np.float32(res.results[0]["out"][0, 0])
    return np.asarray(val, dtype=np.float32)


def _install_ntff_hook():
    """Shim antenv.axon_hooks so trace=True can capture NTFF profiles."""
    import types
    import antenv
    if hasattr(antenv, "axon_hooks"):
        return
    hooks = types.ModuleType("antenv.axon_hooks")
    holder = [None]
    hooks.set_axon_ntff_profile_hook = lambda h: holder.__setitem__(0, h)
    hooks.get_axon_ntff_profile_hook = lambda: holder[0]
    sys.modules["antenv.axon_hooks"] = hooks
    antenv.axon_hooks = hooks
    try:
        from trn_agent_boot.trn_boot import _ntff_profile_via_ctypes
        hooks.set_axon_ntff_profile_hook(
            _ntff_profile_via_ctypes("/opt/axon/libaxon_pjrt.so"))
    except Exception:
        pass



# revision 2
# speedup vs baseline: 2.3759x; 2.3759x over previous
"""Adaptive-softmax NLL loss on 8 TRN2 NeuronCores.

Strategy: tensor-parallel over the vocab dimension, fp8(e4m3) DoubleRow
matmuls (2 fp8 weights per PE cell -> 256-deep contraction per pass).
Each core computes the exp-sums of its vocab slice of head / tail1 /
tail2 logits for all 4096 tokens, plus (token-sharded) the gathered
target-logit dot products. One AllReduce combines per-token sum-exp
partials; every core then finishes the scalar NLL identically.

NLL = sum_n log(S_head_n) + sum_{c1} log(S_t1_n) + sum_{c2} log(S_t2_n)
      - sum_n x_n . W_ext[cidx_n] - sum_{c1} h1_n . W1[t_n-C0] - sum_{c2} h2_n . W2[t_n-C1]

Weights are scaled by 64 on the host before the fp8 cast (keeps values
out of the e4m3 subnormal range); the exp/copy activations undo the
scale with scale=1/64. Logit exps are written to SBUF in bf16 and
summed on the vector engine (2x packed mode), keeping the scalar
engine (the throughput limiter) on pure exp work.
"""

import os
import sys

for _p in ("/opt/trn_rl_repo",):
    if _p not in sys.path:
        sys.path.insert(0, _p)

import ml_dtypes
import numpy as np

import concourse.bacc as bacc
import concourse.bass as bass
import concourse.mybir as mybir
import concourse.tile as tile
from concourse.bass_utils import run_bass_kernel_spmd

dt = mybir.dt
AF = mybir.ActivationFunctionType
ALU = mybir.AluOpType
DR = mybir.MatmulPerfMode.DoubleRow
AXX = mybir.AxisListType.X

NCORES = 8
N, D = 4096, 1024
C0, C1, C2 = 20000, 40000, 50257
VH = C0 + 2          # head logits incl 2 cluster columns
R1, R2 = 256, 64
VHC = 2504           # head vocab cols per core (8*2504 = 20032, pad 30)
V1C = 2500           # tail1 cols per core     (8*2500 = 20000, pad 0)
V2C = 1283           # tail2 cols per core     (8*1283 = 10264, pad 7)
PAD_H = NCORES * VHC - VH
PAD_2 = NCORES * V2C - (C2 - C1)
NT = N // 128        # 32 token tiles
NSH = N // NCORES    # 512 tokens per core for the sharded head dot
WS = 64.0            # host-side weight scale before fp8 cast
INV = 1.0 / WS

F32, BF16, F8 = dt.float32, dt.bfloat16, dt.float8e4
NP_F8 = ml_dtypes.float8_e4m3

# phase-C units: (cluster, base col, width); each <= 1024 (2 PSUM banks)
UNITS = [("H", 0, 1024), ("H", 1024, 1024), ("H", 2048, VHC - 2048),
         ("T1", 0, 1024), ("T1", 1024, 1024), ("T1", 2048, V1C - 2048),
         ("T2", 0, 1024), ("T2", 1024, V2C - 1024)]
NU = len(UNITS)

LAST_EXEC_NS = None


def _build():
    nc = bacc.Bacc("TRN2", target_bir_lowering=False, debug=False,
                   num_devices=NCORES)

    U8 = dt.uint8
    xq_in = nc.declare_dram_parameter("xq", [128, 8 * N], U8, isOutput=False)
    whq_in = nc.declare_dram_parameter("whq", [128, 8 * VHC], U8, isOutput=False)
    w1q_in = nc.declare_dram_parameter("w1q", [128, 2 * V1C], U8, isOutput=False)
    w2q_in = nc.declare_dram_parameter("w2q", [64, V2C], U8, isOutput=False)
    p1q_in = nc.declare_dram_parameter("p1q", [128, 8 * R1], U8, isOutput=False)
    p2q_in = nc.declare_dram_parameter("p2q", [128, 8 * R2], U8, isOutput=False)
    w1sq_in = nc.declare_dram_parameter("w1sq", [128, 2 * N], U8, isOutput=False)
    w2sq_in = nc.declare_dram_parameter("w2sq", [64, N], U8, isOutput=False)
    xTc_in = nc.declare_dram_parameter("xTc", [D, NSH], F32, isOutput=False)
    wselT_in = nc.declare_dram_parameter("wselT", [D, NSH], F32, isOutput=False)
    m1_in = nc.declare_dram_parameter("m1", [128, NT], F32, isOutput=False)
    m2_in = nc.declare_dram_parameter("m2", [128, NT], F32, isOutput=False)
    out_ext = nc.declare_dram_parameter("out", [1, 1], F32, isOutput=True)

    with tile.TileContext(nc) as tc:
        with (
            tc.tile_pool(name="res", bufs=1) as res,
            tc.tile_pool(name="dram", bufs=1, space="DRAM") as dram,
        ):
            # ---- resident loads (critical-path first) ----------------------
            xq_sb = res.tile([128, 8 * N], F8)
            nc.sync.dma_start(out=xq_sb[:], in_=xq_in.ap().bitcast(F8))
            p1q_sb = res.tile([128, 8 * R1], F8)
            nc.sync.dma_start(out=p1q_sb[:], in_=p1q_in.ap().bitcast(F8))
            p2q_sb = res.tile([128, 8 * R2], F8)
            nc.sync.dma_start(out=p2q_sb[:], in_=p2q_in.ap().bitcast(F8))
            whq_sb = res.tile([128, 8 * VHC], F8)
            for c in range(4):
                cs = slice(c * 2 * VHC, (c + 1) * 2 * VHC)
                nc.sync.dma_start(out=whq_sb[:, cs],
                                  in_=whq_in.ap().bitcast(F8)[:, cs])
            w1q_sb = res.tile([128, 2 * V1C], F8)
            nc.sync.dma_start(out=w1q_sb[:], in_=w1q_in.ap().bitcast(F8))
            w2q_sb = res.tile([64, V2C], F8)
            nc.sync.dma_start(out=w2q_sb[:], in_=w2q_in.ap().bitcast(F8))
            w1sq_sb = res.tile([128, 2 * N], F8)
            nc.sync.dma_start(out=w1sq_sb[:], in_=w1sq_in.ap().bitcast(F8))
            w2sq_sb = res.tile([64, N], F8)
            nc.sync.dma_start(out=w2sq_sb[:], in_=w2sq_in.ap().bitcast(F8))
            m1_sb = res.tile([128, NT], F32)
            nc.sync.dma_start(out=m1_sb[:], in_=m1_in.ap())
            m2_sb = res.tile([128, NT], F32)
            nc.sync.dma_start(out=m2_sb[:], in_=m2_in.ap())

            h1q_sb = res.tile([128, 2 * N], F8)
            h2q_sb = res.tile([64, N], F8)
            slots = res.tile([128, NU * NT], F32)
            dsh_slots = res.tile([128, 8], F32)
            dg1_slots = res.tile([128, 16], F32)
            dg2_slots = res.tile([64, 8], F32)

            # DoubleRow-layout views: [128, c-chunk, i, cols]
            xq_c = [xq_sb[:, c * 2 * N:(c + 1) * 2 * N]
                    .rearrange("p (i n) -> p i n", i=2) for c in range(4)]
            whq_c = [whq_sb[:, c * 2 * VHC:(c + 1) * 2 * VHC]
                     .rearrange("p (i v) -> p i v", i=2) for c in range(4)]
            p1q_c = [p1q_sb[:, c * 2 * R1:(c + 1) * 2 * R1]
                     .rearrange("p (i r) -> p i r", i=2) for c in range(4)]
            p2q_c = [p2q_sb[:, c * 2 * R2:(c + 1) * 2 * R2]
                     .rearrange("p (i r) -> p i r", i=2) for c in range(4)]
            w1q_v = w1q_sb[:].rearrange("p (i v) -> p i v", i=2)
            h1q_v = h1q_sb[:].rearrange("p (i n) -> p i n", i=2)
            w1sq_v = w1sq_sb[:].rearrange("p (i n) -> p i n", i=2)

            # ---- phase A: projections h1 = P1 @ x.T, h2 = P2 @ x.T ---------
            with tc.tile_pool(name="pj", bufs=1, space="PSUM") as pj:
                for q in range(8):           # 512-token quarters
                    qs = slice(q * 512, (q + 1) * 512)
                    pa = pj.tile([128, 512], F32, tag="pa", bufs=2)
                    pb = pj.tile([128, 512], F32, tag="pb", bufs=2)
                    pc = pj.tile([64, 512], F32, tag="pc", bufs=2)
                    for c in range(4):
                        st = dict(start=(c == 0), stop=(c == 3))
                        nc.tensor.matmul(pa[:], lhsT=p1q_c[c][:, :, 0:128],
                                         rhs=xq_c[c][:, :, qs],
                                         perf_mode=DR, **st)
                        nc.tensor.matmul(pb[:], lhsT=p1q_c[c][:, :, 128:256],
                                         rhs=xq_c[c][:, :, qs],
                                         perf_mode=DR, **st)
                        nc.tensor.matmul(pc[:], lhsT=p2q_c[c][:, :, 0:64],
                                         rhs=xq_c[c][:, :, qs],
                                         perf_mode=DR, **st)
                    nc.scalar.mul(h1q_v[:, 0, qs], pa[:], INV)
                    nc.scalar.mul(h1q_v[:, 1, qs], pb[:], INV)
                    nc.scalar.mul(h2q_sb[:, qs], pc[:], INV)

            # ---- phase C: head + tail logits, exp(bf16), per-token sums ----
            # tail target-dot chunks (phase A') and the sharded head-dot
            # stream (phase B) are interleaved at tile boundaries so the DVE
            # work rides under the scalar-engine exp stream.
            dve_jobs = []
            for h in range(2):              # tail1 dot: 8 chunks of 1024
                for qq in range(4):
                    dve_jobs.append(("t1", h, qq))
            for qq in range(4):             # tail2 dot: 4 chunks of 1024
                dve_jobs.append(("t2", 0, qq))
            for k in range(8):              # sharded head dot: 8 k-chunks
                dve_jobs.append(("dsh", 0, k))
            nc.vector.memset(dg1_slots[:], 0.0)
            nc.vector.memset(dg2_slots[:], 0.0)

            with (
                tc.tile_pool(name="pp", bufs=4, space="PSUM") as pp,
                tc.tile_pool(name="eb", bufs=4) as ebp,
                tc.tile_pool(name="ds", bufs=2) as dsp,
            ):
                job_i = 0
                for nt in range(NT):
                    nts = slice(nt * 128, (nt + 1) * 128)
                    for u, (cl, base, w) in enumerate(UNITS):
                        pu = pp.tile([128, 1024], F32, tag="pu")
                        for off in range(0, w, 512):
                            cw = min(512, w - off)
                            vs = slice(base + off, base + off + cw)
                            po = pu[:, off:off + cw]
                            if cl == "H":
                                for c in range(4):
                                    nc.tensor.matmul(
                                        po, lhsT=xq_c[c][:, :, nts],
                                        rhs=whq_c[c][:, :, vs], perf_mode=DR,
                                        start=(c == 0), stop=(c == 3))
                            elif cl == "T1":
                                nc.tensor.matmul(
                                    po, lhsT=h1q_v[:, :, nts],
                                    rhs=w1q_v[:, :, vs], perf_mode=DR,
                                    start=True, stop=True)
                            else:
                                nc.tensor.matmul(
                                    po, lhsT=h2q_sb[:, nts],
                                    rhs=w2q_sb[:, vs],
                                    start=True, stop=True)
                        eb = ebp.tile([128, 1024], BF16, tag="eb")
                        nc.scalar.activation(eb[:, 0:w], pu[:, 0:w], AF.Exp,
                                             scale=INV)
                        col = nt * NU + u
                        nc.vector.reduce_sum(slots[:, col:col + 1],
                                             eb[:, 0:w], axis=AXX)
                    # interleave 2 DVE dot jobs per tile starting at tile 8
                    if nt >= 8:
                        for _ in range(2):
                            if job_i < len(dve_jobs):
                                _emit_dot(nc, dsp, dve_jobs[job_i], h1q_v,
                                          w1sq_v, h2q_sb, w2sq_sb, xTc_in,
                                          wselT_in, dg1_slots, dg2_slots,
                                          dsh_slots)
                                job_i += 1
                while job_i < len(dve_jobs):
                    _emit_dot(nc, dsp, dve_jobs[job_i], h1q_v, w1sq_v,
                              h2q_sb, w2sq_sb, xTc_in, wselT_in,
                              dg1_slots, dg2_slots, dsh_slots)
                    job_i += 1

            # ---- phase D: fold unit sums, AllReduce ------------------------
            PAY = 3 * NT + 1
            pay_sb = res.tile([128, PAY], F32)
            slots3 = slots[:].rearrange("p (t u) -> p t u", u=NU)
            nc.vector.reduce_sum(pay_sb[:, 0:NT], slots3[:, :, 0:3], axis=AXX)
            nc.vector.reduce_sum(pay_sb[:, NT:2 * NT], slots3[:, :, 3:6],
                                 axis=AXX)
            nc.vector.reduce_sum(pay_sb[:, 2 * NT:3 * NT], slots3[:, :, 6:8],
                                 axis=AXX)
            nc.vector.reduce_sum(pay_sb[:, 3 * NT:PAY], dsh_slots[:],
                                 axis=AXX)
            pay_dram = dram.tile([128, PAY], F32)
            red_dram = dram.tile([128, PAY], F32)
            nc.sync.dma_start(out=pay_dram[:], in_=pay_sb[:])
            nc.gpsimd.collective_compute(
                "AllReduce", ALU.add,
                replica_groups=[list(range(NCORES))],
                ins=[pay_dram.opt()], outs=[red_dram.opt()])
            red_sb = res.tile([128, PAY], F32)
            nc.sync.dma_start(out=red_sb[:], in_=red_dram[:])

            # ---- phase E: finish scalar NLL (identical on every core) ------
            nc.vector.tensor_scalar_add(red_sb[:, 0:NT], red_sb[:, 0:NT],
                                        float(-PAD_H))
            nc.vector.tensor_scalar_add(red_sb[:, 2 * NT:3 * NT],
                                        red_sb[:, 2 * NT:3 * NT],
                                        float(-PAD_2))
            logs = res.tile([128, 3 * NT], F32)
            nc.scalar.activation(logs[:], red_sb[:, 0:3 * NT], AF.Ln)
            lse = res.tile([128, NT], F32)
            nc.vector.tensor_mul(lse[:], logs[:, NT:2 * NT], m1_sb[:])
            t2m = res.tile([128, NT], F32)
            nc.vector.tensor_mul(t2m[:], logs[:, 2 * NT:3 * NT], m2_sb[:])
            nc.vector.tensor_add(lse[:], lse[:], logs[:, 0:NT])
            nc.vector.tensor_add(lse[:], lse[:], t2m[:])
            tot = res.tile([128, 1], F32)
            nc.vector.reduce_sum(tot[:], lse[:], axis=AXX)
            nc.vector.tensor_sub(tot[:], tot[:], red_sb[:, 3 * NT:PAY])
            dgr = res.tile([128, 1], F32)
            nc.vector.reduce_sum(dgr[:], dg1_slots[:], axis=AXX)
            nc.vector.tensor_scalar_mul(dgr[:], dgr[:], INV)
            nc.vector.tensor_sub(tot[:], tot[:], dgr[:])
            t2r = res.tile([64, 1], F32)
            nc.vector.reduce_sum(t2r[:], dg2_slots[:], axis=AXX)
            nc.vector.tensor_scalar_mul(t2r[:], t2r[:], INV)
            nc.vector.tensor_sub(tot[:64, :], tot[:64, :], t2r[:])
            out_sb = res.tile([1, 1], F32)
            nc.gpsimd.tensor_reduce(out_sb[:], tot[:],
                                    axis=mybir.AxisListType.C, op=ALU.add)
            nc.sync.dma_start(out=out_ext.ap(), in_=out_sb[:])

    nc.compile()
    return nc


def _emit_dot(nc, dsp, job, h1q_v, w1sq_v, h2q_sb, w2sq_sb, xTc_in,
              wselT_in, dg1_slots, dg2_slots, dsh_slots):
    kind, h, k = job
    if kind == "t1":
        ks = slice(k * 1024, (k + 1) * 1024)
        scr = dsp.tile([128, 1024], BF16, tag="scr")
        nc.vector.tensor_mul(scr[:], h1q_v[:, h, ks], w1sq_v[:, h, ks])
        tmp = dsp.tile([128, 1], F32, tag="dtmp")
        nc.vector.reduce_sum(tmp[:], scr[:], axis=AXX)
        col = h * 4 + k
        nc.vector.tensor_add(dg1_slots[:, col:col + 1],
                             dg1_slots[:, col:col + 1], tmp[:])
    elif kind == "t2":
        ks = slice(k * 1024, (k + 1) * 1024)
        scr = dsp.tile([64, 1024], BF16, tag="scr2")
        nc.vector.tensor_mul(scr[:], h2q_sb[:, ks], w2sq_sb[:, ks])
        tmp = dsp.tile([64, 1], F32, tag="dtmp2")
        nc.vector.reduce_sum(tmp[:], scr[:], axis=AXX)
        nc.vector.tensor_add(dg2_slots[:, k:k + 1],
                             dg2_slots[:, k:k + 1], tmp[:])
    else:  # dsh
        xc = dsp.tile([128, NSH], F32, tag="xc")
        nc.sync.dma_start(out=xc[:], in_=xTc_in.ap()[k * 128:(k + 1) * 128, :])
        wc = dsp.tile([128, NSH], F32, tag="wc")
        nc.sync.dma_start(out=wc[:],
                          in_=wselT_in.ap()[k * 128:(k + 1) * 128, :])
        scr = dsp.tile([128, NSH], BF16, tag="scr3")
        nc.vector.tensor_mul(scr[:], xc[:], wc[:])
        nc.vector.reduce_sum(dsh_slots[:, k:k + 1], scr[:], axis=AXX)


_NC = None


def _get_nc():
    global _NC
    if _NC is None:
        _NC = _build()
    return _NC


def _dr_layout_d(mT):
    """[D, cols] -> DoubleRow layout [128, 4*2*cols] for contraction over D."""
    cols = mT.shape[1]
    return np.ascontiguousarray(
        mT.reshape(4, 2, 128, cols).transpose(2, 0, 1, 3).reshape(128, -1))


def _dr_layout_r(mT):
    """[256, cols] -> DoubleRow layout [128, 2*cols] for contraction over R1."""
    cols = mT.shape[1]
    return np.ascontiguousarray(
        mT.reshape(2, 128, cols).transpose(1, 0, 2).reshape(128, -1))


def _prep_inputs(inputs):
    x = np.ascontiguousarray(inputs["x"], dtype=np.float32)
    target = np.asarray(inputs["target"]).astype(np.int64)
    W_head = np.asarray(inputs["W_head"], dtype=np.float32)
    W_cluster = np.asarray(inputs["W_cluster"], dtype=np.float32)
    P1 = np.asarray(inputs["P1"], dtype=np.float32)
    W1 = np.asarray(inputs["W1"], dtype=np.float32)
    P2 = np.asarray(inputs["P2"], dtype=np.float32)
    W2 = np.asarray(inputs["W2"], dtype=np.float32)

    W_ext = np.concatenate([W_head, W_cluster], axis=0)          # [20002, D]
    mask1 = (target >= C0) & (target < C1)
    mask2 = target >= C1
    cidx = np.where(target < C0, target,
                    np.where(mask1, C0, C0 + 1)).astype(np.int64)
    W1p = np.concatenate([WS * W1, np.zeros((1, R1), np.float32)], axis=0)
    W2p = np.concatenate([WS * W2, np.zeros((1, R2), np.float32)], axis=0)
    j1 = np.where(mask1, target - C0, C1 - C0).astype(np.int64)
    j2 = np.where(mask2, target - C1, C2 - C1).astype(np.int64)

    xT = np.ascontiguousarray(x.T)                               # [D, N]
    xq = _dr_layout_d(xT.astype(NP_F8)).view(np.uint8)

    Whfull = np.zeros((NCORES * VHC, D), np.float32)
    Whfull[:VH] = WS * W_ext
    W1full = np.zeros((NCORES * V1C, R1), np.float32)
    W1full[:C1 - C0] = WS * W1
    W2full = np.zeros((NCORES * V2C, R2), np.float32)
    W2full[:C2 - C1] = WS * W2

    p1q = _dr_layout_d((WS * P1).T.astype(NP_F8)).view(np.uint8)
    p2q = _dr_layout_d((WS * P2).T.astype(NP_F8)).view(np.uint8)
    w1sq = _dr_layout_r(W1p[j1].T.astype(NP_F8)).view(np.uint8)
    w2sq = np.ascontiguousarray(W2p[j2].T.astype(NP_F8)).view(np.uint8)
    wselT = np.ascontiguousarray(W_ext[cidx].T)                  # [D, N] f32
    m1 = np.ascontiguousarray(mask1.astype(np.float32).reshape(NT, 128).T)
    m2 = np.ascontiguousarray(mask2.astype(np.float32).reshape(NT, 128).T)

    in_maps = []
    for i in range(NCORES):
        whq = _dr_layout_d(
            np.ascontiguousarray(
                Whfull[i * VHC:(i + 1) * VHC].T).astype(NP_F8)).view(np.uint8)
        w1q = _dr_layout_r(
            np.ascontiguousarray(
                W1full[i * V1C:(i + 1) * V1C].T).astype(NP_F8)).view(np.uint8)
        w2q = np.ascontiguousarray(
            W2full[i * V2C:(i + 1) * V2C].T.astype(NP_F8)).view(np.uint8)
        in_maps.append({
            "xq": xq,
            "whq": whq,
            "w1q": w1q,
            "w2q": w2q,
            "p1q": p1q,
            "p2q": p2q,
            "w1sq": w1sq,
            "w2sq": w2sq,
            "xTc": np.ascontiguousarray(xT[:, i * NSH:(i + 1) * NSH]),
            "wselT": np.ascontiguousarray(wselT[:, i * NSH:(i + 1) * NSH]),
            "m1": m1,
            "m2": m2,
        })
    return in_maps


def kernel(**inputs):
    in_maps = _prep_inputs(inputs)
    nc = _get_nc()
    trace = bool(int(os.environ.get("KERNEL_TRACE", "0")))
    if trace:
        _install_ntff_hook()
    res = run_bass_kernel_spmd(nc, in_maps, core_ids=list(range(NCORES)),
                               trace=trace)
    global LAST_EXEC_NS
    LAST_EXEC_NS = res.exec_time_ns
    val = np.float32(res.results[0]["out"][0, 0])
    return np.asarray(val, dtype=np.float32)


def _install_ntff_hook():
    """Shim antenv.axon_hooks so trace=True can capture NTFF profiles."""
    import types
    import antenv
    if hasattr(antenv, "axon_hooks"):
        return
    hooks = types.ModuleType("antenv.axon_hooks")
    holder = [None]
    hooks.set_axon_ntff_profile_hook = lambda h: holder.__setitem__(0, h)
    hooks.get_axon_ntff_profile_hook = lambda: holder[0]
    sys.modules["antenv.axon_hooks"] = hooks
    antenv.axon_hooks = hooks
    try:
        from trn_agent_boot.trn_boot import _ntff_profile_via_ctypes
        hooks.set_axon_ntff_profile_hook(
            _ntff_profile_via_ctypes("/opt/axon/libaxon_pjrt.so"))
    except Exception:
        pass
